# revision 13
# baseline (speedup 1.0000x reference)
"""Trainium2 Bass kernel for nn_BBoxDetector (conv backbone + decode + NMS).

Contract: kernel(**inputs) takes the FULL unsharded inputs (as produced by
setup_inputs) and returns (det_boxes [16,100,4] f32, det_valid [16,100] bool).

Sharding: data-parallel over batch — 16 images, 8 cores, 2 images per core.
Weights are BN-folded + pre-transposed on the host and replicated.

Numerics: convs in true fp32 on the PE (4 cycles/row); decode transcendentals
via Cody-Waite range reduction + degree-7 Taylor on DVE (≈3e-7 rel) so the
greedy NMS makes bit-identical selections vs the f32 reference.

NMS: per-partition top-24 prefilter (logit ranking == sigmoid-score ranking:
sigmoid is monotone and the output never needs the scores themselves), then
100 greedy iterations over the 126x24 candidate pool entirely on-chip.
"""

import math

import numpy as np

import concourse.bass as bass
import concourse.mybir as mybir
from concourse import bacc
from concourse.tile import TileContext
from concourse.bass_utils import run_bass_kernel_spmd

# ---------------------------------------------------------------- constants
B, C0, H, W = 16, 512, 56, 56
HW = H * W                      # 3136
A = 9
NCAND = A * HW                  # 28224
PP, FF = 126, 224               # score layout: 126 partitions x 224 (= 28224)
POOLW = 24                      # per-partition candidate pool (3 rounds of 8)
MAX_DET = 100
NEG = -1e30
N_CORES = 8
IMGS_PER_CORE = B // N_CORES    # 2
HP, WP = 58, 58                 # padded feature map
HWP = HP * WP                   # 3364
YB = 7                          # y blocks of 8 rows
NB = 8 * W                      # 448 columns per psum tile

INV_LN2 = float(np.float32(1.4426950408889634))
LN2_HI = 0.693359375            # 9-bit mantissa: n*LN2_HI exact
LN2_LO = float(np.float32(-2.1219444005469057e-4))
MAGIC = 12582912.0              # 1.5 * 2^23: round-to-nearest-int trick
R56 = float(np.float32(1.0 / 56.0))
# degree-7 Taylor for exp on [-0.347, 0.347]
EXPC = [1.0 / math.factorial(k) for k in range(8)]

_PROGRAM_CACHE = {}


def _conv_layer(nc, psp, scr_pool, srch, srcl, wth, wtl, bias_t, dsth, dstl,
                n_kt, n_mt, oc, dst_padded, dst32=None):
    """fp16 3-term split 3x3 conv: psum += wh.T@xh + wh.T@xl + wl.T@xh.

    srch/srcl: SBUF fp16 [128, n_kt, 3364] zero-padded hi/lo inputs
    wth/wtl:   SBUF fp16 [128, n_kt, 9, OC] lhsT hi/lo weights
    dsth/dstl: fp16 hi/lo outputs (padded) -- or dst32 f32 (unpadded, conv3)
    """
    taps = [(dy, dx) for dy in range(3) for dx in range(3)]
    sh, sl = [], []
    for kt in range(n_kt):
        sh.append(srch[:, kt, :].rearrange("p (a b) -> p a b", b=WP))
        sl.append(srcl[:, kt, :].rearrange("p (a b) -> p a b", b=WP))
    for mt in range(n_mt):
        mrows = min(128, oc - mt * 128)
        for yb in range(YB):
            ps = psp.tile([mrows, 8, W], mybir.dt.float32, tag="cps")
            n_acc = n_kt * len(taps) * 3
            k = 0
            for kt in range(n_kt):
                for (dy, dx) in taps:
                    rh = sh[kt][:, yb * 8 + dy:yb * 8 + dy + 8, dx:dx + W]
                    rl = sl[kt][:, yb * 8 + dy:yb * 8 + dy + 8, dx:dx + W]
                    lh = wth[:, kt, dy * 3 + dx, mt * 128:mt * 128 + mrows]
                    ll = wtl[:, kt, dy * 3 + dx, mt * 128:mt * 128 + mrows]
                    for (lw, rr) in ((lh, rh), (lh, rl), (ll, rh)):
                        nc.tensor.matmul(ps, lw, rr, start=(k == 0), stop=(k == n_acc - 1))
                        k += 1
            if dst_padded:
                r32 = scr_pool.tile([128, 8, W], mybir.dt.float32, tag="epi32")
                nc.scalar.activation(out=r32[0:mrows], in_=ps[:, :, :],
                                     func=mybir.ActivationFunctionType.Relu,
                                     bias=bias_t[0:mrows, mt:mt + 1])
                dh = dsth[0:mrows, mt, :].rearrange("p (a b) -> p a b", b=WP)[:, yb * 8 + 1:yb * 8 + 9, 1:57]
                dl = dstl[0:mrows, mt, :].rearrange("p (a b) -> p a b", b=WP)[:, yb * 8 + 1:yb * 8 + 9, 1:57]
                nc.vector.tensor_copy(dh, r32[0:mrows])
                nc.vector.tensor_tensor(out=dl, in0=r32[0:mrows], in1=dh,
                                        op=mybir.AluOpType.subtract)
            else:
                out_ap = dst32[0:mrows, yb * NB:(yb + 1) * NB]
                nc.scalar.activation(out=out_ap, in_=ps[:, :, :],
                                     func=mybir.ActivationFunctionType.Relu,
                                     bias=bias_t[0:mrows, mt:mt + 1])


def _conv1x1_out(nc, psp, src, wot, bo_t, dst):
    """convo: 1x1, 64 -> 45 channels."""
    for yb in range(YB):
        ps = psp.tile([45, NB], mybir.dt.float32, tag="cps")
        nc.tensor.matmul(ps, wot[:, :], src[0:64, yb * NB:(yb + 1) * NB],
                         start=True, stop=True)
        nc.vector.tensor_scalar(out=dst[:, yb * NB:(yb + 1) * NB], in0=ps,
                                scalar1=bo_t[:, 0:1], scalar2=None,
                                op0=mybir.AluOpType.add)


def _bc(ap2, mid, last):
    """[P, k] -> [P, mid, k(or bcast last)] with a 0-step broadcast dim."""
    if last is None:
        return bass.AP(tensor=ap2.tensor, offset=ap2.offset,
                       ap=[list(ap2.ap[0]), [0, mid], list(ap2.ap[1])])
    # ap2 is [P, k]: broadcast along a new trailing dim of size `last`
    return bass.AP(tensor=ap2.tensor, offset=ap2.offset,
                   ap=[list(ap2.ap[0]), list(ap2.ap[1]), [0, last]])


def _exp_poly(nc, pool, z, e_out):
    """e_out = exp(z) elementwise, ~3e-7 rel accuracy. z, e_out: [126, n, 24]."""
    f32 = mybir.dt.float32
    shp = list(z.shape)
    t = pool.tile(shp, f32, tag="xp_t")
    nc.vector.tensor_scalar(out=t, in0=z, scalar1=INV_LN2, scalar2=MAGIC,
                            op0=mybir.AluOpType.mult, op1=mybir.AluOpType.add)
    nf = pool.tile(shp, f32, tag="xp_nf")
    nc.vector.tensor_scalar(out=nf, in0=t, scalar1=MAGIC, scalar2=None,
                            op0=mybir.AluOpType.subtract)
    r = pool.tile(shp, f32, tag="xp_r")
    nc.vector.scalar_tensor_tensor(out=r, in0=nf, scalar=-LN2_HI, in1=z,
                                   op0=mybir.AluOpType.mult, op1=mybir.AluOpType.add)
    nc.vector.scalar_tensor_tensor(out=r, in0=nf, scalar=-LN2_LO, in1=r,
                                   op0=mybir.AluOpType.mult, op1=mybir.AluOpType.add)
    # Horner: p = ((C7*r + C6)*r + C5)...
    p = pool.tile(shp, f32, tag="xp_p")
    nc.vector.tensor_scalar(out=p, in0=r, scalar1=EXPC[7], scalar2=EXPC[6],
                            op0=mybir.AluOpType.mult, op1=mybir.AluOpType.add)
    for k in range(5, -1, -1):
        nc.vector.tensor_tensor(out=p, in0=p, in1=r, op=mybir.AluOpType.mult)
        nc.vector.tensor_scalar_add(p, p, EXPC[k])
    # 2^n via exponent-field construction
    u = pool.tile(shp, f32, tag="xp_u")
    nc.vector.tensor_scalar(out=u, in0=nf, scalar1=127.0, scalar2=8388608.0,
                            op0=mybir.AluOpType.add, op1=mybir.AluOpType.mult)
    ui = pool.tile(shp, mybir.dt.int32, tag="xp_ui")
    nc.vector.tensor_copy(ui, u)
    nc.vector.tensor_tensor(out=e_out, in0=p, in1=ui[:, :, :].bitcast(f32),
                            op=mybir.AluOpType.mult)


def _build_program(debug=False):
    f32 = mybir.dt.float32
    u32 = mybir.dt.uint32
    Relu = mybir.ActivationFunctionType.Relu
    nc = bacc.Bacc()

    f16 = mybir.dt.float16
    featsh = nc.dram_tensor("featsh", [IMGS_PER_CORE, 128, 4, HWP], f16, kind="ExternalInput")
    featsl = nc.dram_tensor("featsl", [IMGS_PER_CORE, 128, 4, HWP], f16, kind="ExternalInput")
    w1th = nc.dram_tensor("w1th", [128, 4 * 9 * 256], f16, kind="ExternalInput")
    w1tl = nc.dram_tensor("w1tl", [128, 4 * 9 * 256], f16, kind="ExternalInput")
    w2th = nc.dram_tensor("w2th", [128, 2 * 9 * 128], f16, kind="ExternalInput")
    w2tl = nc.dram_tensor("w2tl", [128, 2 * 9 * 128], f16, kind="ExternalInput")
    w3th = nc.dram_tensor("w3th", [128, 1 * 9 * 64], f16, kind="ExternalInput")
    w3tl = nc.dram_tensor("w3tl", [128, 1 * 9 * 64], f16, kind="ExternalInput")
    wot = nc.dram_tensor("wot", [64, 45], f32, kind="ExternalInput")
    b1d = nc.dram_tensor("b1d", [128, 2], f32, kind="ExternalInput")
    b2d = nc.dram_tensor("b2d", [128, 1], f32, kind="ExternalInput")
    b3d = nc.dram_tensor("b3d", [64, 1], f32, kind="ExternalInput")
    bod = nc.dram_tensor("bod", [45, 1], f32, kind="ExternalInput")
    pmd = nc.dram_tensor("pmd", [PP, 1], f32, kind="ExternalInput")
    awd = nc.dram_tensor("awd", [PP, 1], f32, kind="ExternalInput")
    ahd = nc.dram_tensor("ahd", [PP, 1], f32, kind="ExternalInput")
    identd = nc.dram_tensor("identd", [128, 128], f32, kind="ExternalInput")

    boxes_o = nc.dram_tensor("boxes_o", [IMGS_PER_CORE, 4 * MAX_DET], f32, kind="ExternalOutput")
    gmlog_o = nc.dram_tensor("gmlog_o", [IMGS_PER_CORE, MAX_DET], f32, kind="ExternalOutput")
    if debug:
        p45_o = nc.dram_tensor("p45_o", [IMGS_PER_CORE, 45, HW], f32, kind="ExternalOutput")
        pool_o = nc.dram_tensor("pool_o", [IMGS_PER_CORE, PP, POOLW], f32, kind="ExternalOutput")
        gi_o = nc.dram_tensor("gi_o", [IMGS_PER_CORE, PP, POOLW], u32, kind="ExternalOutput")
        x9_o = nc.dram_tensor("x9_o", [IMGS_PER_CORE, PP, 9, POOLW], f32, kind="ExternalOutput")

    with TileContext(nc) as tc:
        with (
            tc.tile_pool(name="wp", bufs=1) as wp,
            tc.tile_pool(name="act", bufs=1) as actp,
            tc.tile_pool(name="nms", bufs=2) as nmsp,
            tc.tile_pool(name="xp", bufs=2) as xpp,
            tc.tile_pool(name="cpsp", bufs=4, space="PSUM") as cpsp,
            tc.tile_pool(name="npsp", bufs=2, space="PSUM") as npsp,
            tc.tile_pool(name="drp", bufs=1, space="DRAM") as drp,
        ):
            # ---- resident constants / weights (fp16 hi/lo)
            w1h = wp.tile([128, 4, 9, 256], f16)
            nc.sync.dma_start(out=w1h, in_=w1th[:, :].rearrange("p (k t o) -> p k t o", k=4, t=9))
            w1l = wp.tile([128, 4, 9, 256], f16)
            nc.sync.dma_start(out=w1l, in_=w1tl[:, :].rearrange("p (k t o) -> p k t o", k=4, t=9))
            w2h = wp.tile([128, 2, 9, 128], f16)
            nc.sync.dma_start(out=w2h, in_=w2th[:, :].rearrange("p (k t o) -> p k t o", k=2, t=9))
            w2l = wp.tile([128, 2, 9, 128], f16)
            nc.sync.dma_start(out=w2l, in_=w2tl[:, :].rearrange("p (k t o) -> p k t o", k=2, t=9))
            w3h = wp.tile([128, 1, 9, 64], f16)
            nc.sync.dma_start(out=w3h, in_=w3th[:, :].rearrange("p (k t o) -> p k t o", k=1, t=9))
            w3l = wp.tile([128, 1, 9, 64], f16)
            nc.sync.dma_start(out=w3l, in_=w3tl[:, :].rearrange("p (k t o) -> p k t o", k=1, t=9))
            wo = wp.tile([64, 45], f32)
            nc.sync.dma_start(out=wo, in_=wot[:, :])
            b1 = wp.tile([128, 2], f32)
            nc.sync.dma_start(out=b1, in_=b1d[:, :])
            b2 = wp.tile([128, 1], f32)
            nc.sync.dma_start(out=b2, in_=b2d[:, :])
            b3 = wp.tile([64, 1], f32)
            nc.sync.dma_start(out=b3, in_=b3d[:, :])
            bo = wp.tile([45, 1], f32)
            nc.sync.dma_start(out=bo, in_=bod[:, :])
            ident = wp.tile([128, 128], f32)
            nc.sync.dma_start(out=ident, in_=identd[:, :])
            pm224 = wp.tile([PP, 1], f32)
            nc.sync.dma_start(out=pm224, in_=pmd[:, :])
            awc = wp.tile([PP, 1], f32)
            nc.sync.dma_start(out=awc, in_=awd[:, :])
            ahc = wp.tile([PP, 1], f32)
            nc.sync.dma_start(out=ahc, in_=ahd[:, :])

            negt = wp.tile([PP, FF], f32)
            nc.vector.memset(negt, NEG)
            ones_col = wp.tile([PP, 1], f32)
            nc.vector.memset(ones_col, 1.0)
            ones_row = wp.tile([1, 128], f32)
            nc.vector.memset(ones_row, 1.0)
            ones_sq = wp.tile([PP, PP], f32)
            nc.vector.memset(ones_sq, 1.0)
            iota = wp.tile([PP, POOLW], u32)
            nc.gpsimd.iota(iota, pattern=[[0, POOLW]], base=0, channel_multiplier=FF)

            # ---- big activation tiles (shared across images; serial on PE)
            feath = actp.tile([128, 4, HWP], f16)
            featl = actp.tile([128, 4, HWP], f16)
            c1h = actp.tile([128, 2, HWP], f16)
            c1l = actp.tile([128, 2, HWP], f16)
            c2h = actp.tile([128, 1, HWP], f16)
            c2l = actp.tile([128, 1, HWP], f16)
            c3 = actp.tile([64, HW], f32)
            p45 = actp.tile([45, HW], f32)
            # zero the pad borders once (epilogues only ever write interiors)
            nc.vector.memset(c1h, 0.0)
            nc.vector.memset(c1l, 0.0)
            nc.vector.memset(c2h, 0.0)
            nc.vector.memset(c2l, 0.0)

            tbl = drp.tile([NCAND, 4], f32, tag="tbl")

            for img in range(IMGS_PER_CORE):
                # ---- convs (fp16 3-term split)
                nc.sync.dma_start(out=feath, in_=featsh[img])
                nc.sync.dma_start(out=featl, in_=featsl[img])
                _conv_layer(nc, cpsp, xpp, feath[:, :, :], featl[:, :, :],
                            w1h[:, :, :, :], w1l[:, :, :, :], b1,
                            c1h[:, :, :], c1l[:, :, :],
                            n_kt=4, n_mt=2, oc=256, dst_padded=True)
                _conv_layer(nc, cpsp, xpp, c1h[:, :, :], c1l[:, :, :],
                            w2h[:, :, :, :], w2l[:, :, :, :], b2,
                            c2h[:, :, :], c2l[:, :, :],
                            n_kt=2, n_mt=1, oc=128, dst_padded=True)
                _conv_layer(nc, cpsp, xpp, c2h[:, :, :], c2l[:, :, :],
                            w3h[:, :, :, :], w3l[:, :, :, :], b3,
                            None, None,
                            n_kt=1, n_mt=1, oc=64, dst_padded=False, dst32=c3)
                _conv1x1_out(nc, cpsp, c3, wo, bo, p45)
                if debug:
                    nc.sync.dma_start(out=p45_o[img], in_=p45[:, :])

                # ---- write box-logit planes into the interleaved DRAM table
                for k in range(4):
                    nc.sync.dma_start(
                        out=tbl[:, k:k + 1],
                        in_=p45[9 * k:9 * (k + 1), :],
                    )

                # ---- score layout [126, 224] + conf filter (logit > 0)
                sfa = nmsp.tile([PP, FF], f32, tag="sfa")
                nc.sync.dma_start(out=sfa[:, :],
                                  in_=p45[36:45, :].rearrange("p (a b) -> p a b", b=FF))
                mneg = nmsp.tile([PP, FF], u32, tag="mneg")
                nc.vector.tensor_scalar(out=mneg, in0=sfa, scalar1=0.0, scalar2=None,
                                        op0=mybir.AluOpType.is_le)
                nc.vector.copy_predicated(sfa, mneg, negt[0:PP, :])

                # ---- per-partition top-24 pool (3 rounds of top-8)
                sp = nmsp.tile([PP, POOLW], f32, tag="sp")
                ii = nmsp.tile([PP, POOLW], u32, tag="ii")
                sfb = nmsp.tile([PP, FF], f32, tag="sfb")
                src = sfa
                for rnd in range(3):
                    nc.vector.max_with_indices(out_max=sp[:, 8 * rnd:8 * (rnd + 1)],
                                               out_indices=ii[:, 8 * rnd:8 * (rnd + 1)],
                                               in_=src)
                    if rnd < 2:
                        dst = sfb if rnd == 0 else sfa
                        nc.vector.match_replace(out=dst, in_to_replace=sp[:, 8 * rnd:8 * (rnd + 1)],
                                                in_values=src, imm_value=NEG)
                        src = dst
                gi = nmsp.tile([PP, POOLW], u32, tag="gi")
                nc.vector.tensor_tensor(out=gi, in0=ii, in1=iota, op=mybir.AluOpType.add)

                # ---- gather tx,ty,tw,th rows for the pool (run-of-4 per row)
                g4 = nmsp.tile([PP, POOLW, 4], f32, tag="g4")
                for c in range(POOLW):
                    nc.gpsimd.indirect_dma_start(
                        out=g4[:, c, :], out_offset=None, in_=tbl[:, :],
                        in_offset=bass.IndirectOffsetOnAxis(ap=gi[:, c:c + 1], axis=0))
                # ---- derive gx, gy arithmetically: hw = (p % 14)*224 + f
                iif = nmsp.tile([PP, POOLW], f32, tag="iif")
                nc.vector.tensor_copy(iif, ii)
                hwf = nmsp.tile([PP, POOLW], f32, tag="hwf")
                nc.vector.tensor_scalar(out=hwf, in0=iif, scalar1=pm224, scalar2=None,
                                        op0=mybir.AluOpType.add)
                gyt = nmsp.tile([PP, POOLW], f32, tag="gyt")
                nc.vector.tensor_scalar(out=gyt, in0=hwf, scalar1=R56, scalar2=-0.499,
                                        op0=mybir.AluOpType.mult, op1=mybir.AluOpType.add)
                nc.vector.tensor_scalar_add(gyt, gyt, MAGIC)
                nc.vector.tensor_scalar(out=gyt, in0=gyt, scalar1=MAGIC, scalar2=None,
                                        op0=mybir.AluOpType.subtract)
                gxt = nmsp.tile([PP, POOLW], f32, tag="gxt")
                nc.vector.scalar_tensor_tensor(out=gxt, in0=gyt, scalar=-56.0, in1=hwf,
                                               op0=mybir.AluOpType.mult,
                                               op1=mybir.AluOpType.add)
                if debug:
                    nc.sync.dma_start(out=pool_o[img], in_=sp[:, :])
                    nc.sync.dma_start(out=gi_o[img], in_=gi[:, :])

                # ---- decode: exp/sigmoid + box geometry
                x9 = nmsp.tile([PP, 9, POOLW], f32, tag="x9")
                z = xpp.tile([PP, 4, POOLW], f32, tag="z")
                gtx = g4[:, :, :].rearrange("p w c -> p c w")
                nc.vector.tensor_scalar(out=z[:, 0:2, :], in0=gtx[:, 0:2, :],
                                        scalar1=-1.0, scalar2=None, op0=mybir.AluOpType.mult)
                nc.vector.tensor_copy(z[:, 2:4, :], gtx[:, 2:4, :])
                ez = xpp.tile([PP, 4, POOLW], f32, tag="ez")
                _exp_poly(nc, xpp, z[:, :, :], ez[:, :, :])
                # sigmoid for x,y
                d = xpp.tile([PP, 2, POOLW], f32, tag="d")
                nc.vector.tensor_scalar_add(d, ez[:, 0:2, :], 1.0)
                sg = xpp.tile([PP, 2, POOLW], f32, tag="sg")
                nc.vector.reciprocal(sg, d)
                # px,py = (sig + g)/56
                nc.vector.tensor_tensor(out=x9[:, 0, :], in0=sg[:, 0, :], in1=gxt,
                                        op=mybir.AluOpType.add)
                nc.vector.tensor_tensor(out=x9[:, 1, :], in0=sg[:, 1, :], in1=gyt,
                                        op=mybir.AluOpType.add)
                nc.vector.tensor_scalar(out=x9[:, 0:2, :], in0=x9[:, 0:2, :], scalar1=R56,
                                        scalar2=None, op0=mybir.AluOpType.mult)
                # pw,ph = min(exp * anc, 1.0), per-partition anchor scalars
                nc.vector.tensor_scalar(out=x9[:, 2, :], in0=ez[:, 2, :], scalar1=awc,
                                        scalar2=1.0, op0=mybir.AluOpType.mult,
                                        op1=mybir.AluOpType.min)
                nc.vector.tensor_scalar(out=x9[:, 3, :], in0=ez[:, 3, :], scalar1=ahc,
                                        scalar2=1.0, op0=mybir.AluOpType.mult,
                                        op1=mybir.AluOpType.min)
                # x1,y1 / -(x2,y2) / area
                h2 = xpp.tile([PP, 2, POOLW], f32, tag="h2")
                nc.vector.tensor_scalar(out=h2, in0=x9[:, 2:4, :], scalar1=0.5,
                                        scalar2=None, op0=mybir.AluOpType.mult)
                nc.vector.tensor_tensor(out=x9[:, 4:6, :], in0=x9[:, 0:2, :], in1=h2,
                                        op=mybir.AluOpType.subtract)
                t2 = xpp.tile([PP, 2, POOLW], f32, tag="t2")
                nc.vector.tensor_tensor(out=t2, in0=x9[:, 0:2, :], in1=h2,
                                        op=mybir.AluOpType.add)
                nc.vector.tensor_scalar(out=x9[:, 6:8, :], in0=t2, scalar1=-1.0,
                                        scalar2=None, op0=mybir.AluOpType.mult)
                e2 = xpp.tile([PP, 2, POOLW], f32, tag="e2")
                nc.vector.tensor_tensor(out=e2, in0=t2, in1=x9[:, 4:6, :],
                                        op=mybir.AluOpType.subtract)
                nc.vector.tensor_tensor(out=x9[:, 8, :], in0=e2[:, 0, :], in1=e2[:, 1, :],
                                        op=mybir.AluOpType.mult)
                if debug:
                    nc.sync.dma_start(out=x9_o[img], in_=x9[:, :, :])

                # ---- greedy NMS, 100 iterations
                out4 = nmsp.tile([1, 4 * MAX_DET], f32, tag="out4")
                gmlog = nmsp.tile([1, MAX_DET], f32, tag="gmlog")
                for it in range(MAX_DET):
                    ips = npsp.tile([128, 140], f32, tag="ips")
                    mT = ips[0:1, 0:PP]
                    gmB = ips[0:PP, 126:127]
                    w9b = ips[0:PP, 128:137]
                    mcol = nmsp.tile([PP, 1], f32, tag="mcol")
                    nc.vector.tensor_reduce(out=mcol, in_=sp, axis=mybir.AxisListType.X,
                                            op=mybir.AluOpType.max)
                    nc.tensor.transpose(mT, mcol, ident[0:PP, 0:PP])
                    nc.vector.tensor_reduce(out=gmlog[0:1, it:it + 1], in_=mT,
                                            axis=mybir.AxisListType.X, op=mybir.AluOpType.max)
                    nc.tensor.matmul(gmB, ones_row[:, 0:PP], gmlog[0:1, it:it + 1],
                                     start=True, stop=True)
                    oh = nmsp.tile([PP, POOLW], f32, tag="oh")
                    nc.vector.tensor_scalar(out=oh, in0=sp, scalar1=gmB, scalar2=None,
                                            op0=mybir.AluOpType.is_equal)
                    t9 = nmsp.tile([PP, 9, POOLW], f32, tag="t9")
                    nc.vector.tensor_tensor(out=t9, in0=x9,
                                            in1=_bc(oh[:, :], 9, None),
                                            op=mybir.AluOpType.mult)
                    r9 = nmsp.tile([PP, 9], f32, tag="r9")
                    nc.vector.tensor_reduce(out=r9, in_=t9, axis=mybir.AxisListType.X,
                                            op=mybir.AluOpType.add)
                    nc.tensor.matmul(w9b, ones_sq, r9, start=True, stop=True)
                    # suppression: M4 = max(candidate [x1,y1,-x2,-y2], winner's)
                    m4 = nmsp.tile([PP, 4, POOLW], f32, tag="m4")
                    nc.vector.tensor_tensor(out=m4, in0=x9[:, 4:8, :],
                                            in1=_bc(w9b[:, 4:8], 0, POOLW),
                                            op=mybir.AluOpType.max)
                    iw = nmsp.tile([PP, POOLW], f32, tag="iw")
                    nc.vector.scalar_tensor_tensor(out=iw, in0=m4[:, 2, :], scalar=-1.0,
                                                   in1=m4[:, 0, :], op0=mybir.AluOpType.mult,
                                                   op1=mybir.AluOpType.subtract)
                    ih = nmsp.tile([PP, POOLW], f32, tag="ih")
                    nc.vector.scalar_tensor_tensor(out=ih, in0=m4[:, 3, :], scalar=-1.0,
                                                   in1=m4[:, 1, :], op0=mybir.AluOpType.mult,
                                                   op1=mybir.AluOpType.subtract)
                    nc.vector.tensor_scalar(out=ih, in0=ih, scalar1=0.0, scalar2=None,
                                            op0=mybir.AluOpType.max)
                    inter = nmsp.tile([PP, POOLW], f32, tag="inter")
                    nc.vector.scalar_tensor_tensor(out=inter, in0=iw, scalar=0.0, in1=ih,
                                                   op0=mybir.AluOpType.max,
                                                   op1=mybir.AluOpType.mult)
                    # supp <=> 0.5*(area+areaw-inter+1e-16) < inter <=> area+areaw+1e-16 < 3*inter
                    uni = nmsp.tile([PP, POOLW], f32, tag="uni")
                    nc.vector.tensor_scalar(out=uni, in0=x9[:, 8, :], scalar1=w9b[:, 8:9],
                                            scalar2=1e-16, op0=mybir.AluOpType.add,
                                            op1=mybir.AluOpType.add)
                    supp = nmsp.tile([PP, POOLW], u32, tag="supp")
                    nc.vector.scalar_tensor_tensor(out=supp, in0=inter, scalar=3.0,
                                                   in1=uni, op0=mybir.AluOpType.mult,
                                                   op1=mybir.AluOpType.is_gt)
                    nc.vector.copy_predicated(sp, supp, negt[0:PP, 0:POOLW])
                    nc.vector.tensor_copy(out4[0:1, 4 * it:4 * (it + 1)], w9b[0:1, 0:4])

                nc.sync.dma_start(out=boxes_o[img:img+1, :], in_=out4[:, :])
                nc.sync.dma_start(out=gmlog_o[img:img+1, :], in_=gmlog[:, :])

    nc.compile()
    return nc


def _host_prep(inputs):
    f = {k: np.ascontiguousarray(np.asarray(v, dtype=np.float32)) for k, v in inputs.items()}

    def fold(w, b, g, be, m, v):
        inv = (g.astype(np.float64) / np.sqrt(v.astype(np.float64) + 1e-5)).astype(np.float32)
        wf = (w * inv[:, None, None, None]).astype(np.float32)
        bf = (b * inv + be - m * inv).astype(np.float32)
        return wf, bf

    w1f, b1f = fold(f["w1"], f["b1"], f["g1"], f["be1"], f["m1"], f["v1"])
    w2f, b2f = fold(f["w2"], f["b2"], f["g2"], f["be2"], f["m2"], f["v2"])
    w3f, b3f = fold(f["w3"], f["b3"], f["g3"], f["be3"], f["m3"], f["v3"])

    def lhsT(wf, n_kt, oc):
        arr = wf.reshape(oc, n_kt, 128, 9)          # [o, kt, ip, tap]
        arr = np.transpose(arr, (2, 1, 3, 0))        # [ip, kt, tap, o]
        arr = np.ascontiguousarray(arr.reshape(128, n_kt * 9 * oc))
        h = arr.astype(np.float16)
        l = (arr - h.astype(np.float32)).astype(np.float16)
        return h, l

    w1Th, w1Tl = lhsT(w1f, 4, 256)
    w2Th, w2Tl = lhsT(w2f, 2, 128)
    w3Th, w3Tl = lhsT(w3f, 1, 64)

    wo = f["wo"][:, :, 0, 0]                         # [54, 64]
    woT = np.zeros((64, 45), np.float32)
    boP = np.zeros((45, 1), np.float32)
    for c in range(5):
        for a in range(A):
            woT[:, c * 9 + a] = wo[a * 6 + c]
            boP[c * 9 + a, 0] = f["bo"][a * 6 + c]

    b1P = np.ascontiguousarray(b1f.reshape(2, 128).T)         # [128, 2]
    b2P = np.ascontiguousarray(b2f.reshape(1, 128).T)
    b3P = b3f.reshape(64, 1).copy()

    feats_pad = np.zeros((B, 4, 128, HP, WP), np.float32)
    feats_pad[:, :, :, 1:57, 1:57] = f["features"].reshape(B, 4, 128, H, W)
    feats_pad = np.ascontiguousarray(
        feats_pad.reshape(B, 4, 128, HWP).transpose(0, 2, 1, 3))
    feats_h = feats_pad.astype(np.float16)
    feats_l = (feats_pad - feats_h.astype(np.float32)).astype(np.float16)

    # per-partition constants: partition p covers i in [p*224, (p+1)*224),
    # anchor a = p//14, hw-offset = (p%14)*224
    pidx = np.arange(PP)
    anc = np.array([[np.float32(s * math.sqrt(r) / 224.0),
                     np.float32(s / math.sqrt(r) / 224.0)]
                    for s in [32.0, 64.0, 128.0] for r in [0.5, 1.0, 2.0]], np.float32)
    pm = ((pidx % 14) * 224).astype(np.float32)[:, None]
    aw = anc[pidx // 14, 0][:, None].copy()
    ah = anc[pidx // 14, 1][:, None].copy()

    shared = {
        "w1th": w1Th, "w1tl": w1Tl, "w2th": w2Th, "w2tl": w2Tl,
        "w3th": w3Th, "w3tl": w3Tl, "wot": woT,
        "b1d": b1P, "b2d": b2P, "b3d": b3P, "bod": boP,
        "pmd": pm, "awd": aw, "ahd": ah,
        "identd": np.eye(128, dtype=np.float32),
    }
    in_maps = []
    for c in range(N_CORES):
        m = dict(shared)
        m["featsh"] = np.ascontiguousarray(feats_h[c * IMGS_PER_CORE:(c + 1) * IMGS_PER_CORE])
        m["featsl"] = np.ascontiguousarray(feats_l[c * IMGS_PER_CORE:(c + 1) * IMGS_PER_CORE])
        in_maps.append(m)
    return in_maps


def kernel(debug=False, trace=False, **inputs):
    key = bool(debug)
    if key not in _PROGRAM_CACHE:
        _PROGRAM_CACHE[key] = _build_program(debug=debug)
    nc = _PROGRAM_CACHE[key]
    in_maps = _host_prep(inputs)
    out = run_bass_kernel_spmd(nc, in_maps, core_ids=list(range(N_CORES)), trace=trace)
    res = out.results
    det_boxes = np.zeros((B, MAX_DET, 4), np.float32)
    det_valid = np.zeros((B, MAX_DET), bool)
    dbg = []
    for c in range(N_CORES):
        r = res[c]
        bx = r["boxes_o"].reshape(IMGS_PER_CORE, MAX_DET, 4)
        gm = r["gmlog_o"]
        ok = gm > np.float32(-5e29)
        bx = np.where(ok[:, :, None], bx, 0.0).astype(np.float32)
        det_boxes[c * IMGS_PER_CORE:(c + 1) * IMGS_PER_CORE] = bx
        det_valid[c * IMGS_PER_CORE:(c + 1) * IMGS_PER_CORE] = ok
        if debug:
            dbg.append(r)
    if debug:
        kernel.debug_results = dbg
        kernel.exec_time_ns = out.exec_time_ns
        return det_boxes, det_valid
    if trace:
        kernel.exec_time_ns = out.exec_time_ns
    return det_boxes, det_valid


# revision 17
# speedup vs baseline: 1.0553x; 1.0553x over previous
"""Trainium2 Bass kernel for nn_BBoxDetector (conv backbone + decode + NMS).

Contract: kernel(**inputs) takes the FULL unsharded inputs (as produced by
setup_inputs) and returns (det_boxes [16,100,4] f32, det_valid [16,100] bool).

Sharding: data-parallel over batch — 16 images, 8 cores, 2 images per core.
Weights are BN-folded + pre-transposed on the host and replicated.

Numerics: convs as fp16 hi/lo 3-term splits (wh.xh+wh.xl+wl.xh, 3 cycles/row,
~1e-5 maxabs logit error); decode transcendentals via Cody-Waite range
reduction + degree-7 Taylor on DVE (~3e-7 rel) so the greedy NMS makes
bit-identical selections vs the f32 reference.

NMS: per-partition top-16 prefilter (logit ranking == sigmoid-score ranking:
sigmoid is monotone and the output never needs the scores themselves), then
100 greedy iterations over the 126x16 candidate pool entirely on-chip.
"""

import math

import numpy as np

import concourse.bass as bass
import concourse.mybir as mybir
from concourse import bacc
from concourse.tile import TileContext
from concourse.bass_utils import run_bass_kernel_spmd

# ---------------------------------------------------------------- constants
B, C0, H, W = 16, 512, 56, 56
HW = H * W                      # 3136
A = 9
NCAND = A * HW                  # 28224
PP, FF = 126, 224               # score layout: 126 partitions x 224 (= 28224)
POOLW = 16                      # per-partition candidate pool (2 rounds of 8)
MAX_DET = 100
NEG = -1e30
N_CORES = 8
IMGS_PER_CORE = B // N_CORES    # 2
HP, WP = 58, 58                 # padded feature map
HWP = HP * WP                   # 3364
YB = 7                          # y blocks of 8 rows
NB = 8 * W                      # 448 columns per psum tile

INV_LN2 = float(np.float32(1.4426950408889634))
LN2_HI = 0.693359375            # 9-bit mantissa: n*LN2_HI exact
LN2_LO = float(np.float32(-2.1219444005469057e-4))
MAGIC = 12582912.0              # 1.5 * 2^23: round-to-nearest-int trick
R56 = float(np.float32(1.0 / 56.0))
# degree-7 Taylor for exp on [-0.347, 0.347]
EXPC = [1.0 / math.factorial(k) for k in range(8)]

_PROGRAM_CACHE = {}


def _conv_layer(nc, psp, scr_pool, srch, srcl, wth, wtl, bias_t, dsth, dstl,
                n_kt, n_mt, oc, dst_padded, dst32=None):
    """fp16 3-term split 3x3 conv: psum += wh.T@xh + wh.T@xl + wl.T@xh.

    srch/srcl: SBUF fp16 [128, n_kt, 3364] zero-padded hi/lo inputs
    wth/wtl:   SBUF fp16 [128, n_kt, 9, OC] lhsT hi/lo weights
    dsth/dstl: fp16 hi/lo outputs (padded) -- or dst32 f32 (unpadded, conv3)
    """
    taps = [(dy, dx) for dy in range(3) for dx in range(3)]
    sh, sl = [], []
    for kt in range(n_kt):
        sh.append(srch[:, kt, :].rearrange("p (a b) -> p a b", b=WP))
        sl.append(srcl[:, kt, :].rearrange("p (a b) -> p a b", b=WP))
    for mt in range(n_mt):
        mrows = min(128, oc - mt * 128)
        for yb in range(YB):
            ps = psp.tile([mrows, 8, W], mybir.dt.float32, tag="cps")
            n_acc = n_kt * len(taps) * 3
            k = 0
            for kt in range(n_kt):
                for (dy, dx) in taps:
                    rh = sh[kt][:, yb * 8 + dy:yb * 8 + dy + 8, dx:dx + W]
                    rl = sl[kt][:, yb * 8 + dy:yb * 8 + dy + 8, dx:dx + W]
                    lh = wth[:, kt, dy * 3 + dx, mt * 128:mt * 128 + mrows]
                    ll = wtl[:, kt, dy * 3 + dx, mt * 128:mt * 128 + mrows]
                    for (lw, rr) in ((lh, rh), (lh, rl), (ll, rh)):
                        nc.tensor.matmul(ps, lw, rr, start=(k == 0), stop=(k == n_acc - 1))
                        k += 1
            if dst_padded:
                r32 = scr_pool.tile([128, 8, W], mybir.dt.float32, tag="epi32")
                nc.scalar.activation(out=r32[0:mrows], in_=ps[:, :, :],
                                     func=mybir.ActivationFunctionType.Relu,
                                     bias=bias_t[0:mrows, mt:mt + 1])
                dh = dsth[0:mrows, mt, :].rearrange("p (a b) -> p a b", b=WP)[:, yb * 8 + 1:yb * 8 + 9, 1:57]
                dl = dstl[0:mrows, mt, :].rearrange("p (a b) -> p a b", b=WP)[:, yb * 8 + 1:yb * 8 + 9, 1:57]
                nc.vector.tensor_copy(dh, r32[0:mrows])
                nc.vector.tensor_tensor(out=dl, in0=r32[0:mrows], in1=dh,
                                        op=mybir.AluOpType.subtract)
            else:
                out_ap = dst32[0:mrows, yb * NB:(yb + 1) * NB]
                nc.scalar.activation(out=out_ap, in_=ps[:, :, :],
                                     func=mybir.ActivationFunctionType.Relu,
                                     bias=bias_t[0:mrows, mt:mt + 1])


def _conv1x1_out(nc, psp, src, wot, bo_t, dst):
    """convo: 1x1, 64 -> 45 channels."""
    for yb in range(YB):
        ps = psp.tile([45, NB], mybir.dt.float32, tag="cps")
        nc.tensor.matmul(ps, wot[:, :], src[0:64, yb * NB:(yb + 1) * NB],
                         start=True, stop=True)
        nc.vector.tensor_scalar(out=dst[:, yb * NB:(yb + 1) * NB], in0=ps,
                                scalar1=bo_t[:, 0:1], scalar2=None,
                                op0=mybir.AluOpType.add)


def _bc(ap2, mid, last):
    """[P, k] -> [P, mid, k(or bcast last)] with a 0-step broadcast dim."""
    if last is None:
        return bass.AP(tensor=ap2.tensor, offset=ap2.offset,
                       ap=[list(ap2.ap[0]), [0, mid], list(ap2.ap[1])])
    # ap2 is [P, k]: broadcast along a new trailing dim of size `last`
    return bass.AP(tensor=ap2.tensor, offset=ap2.offset,
                   ap=[list(ap2.ap[0]), list(ap2.ap[1]), [0, last]])


def _exp_poly(nc, pool, z, e_out):
    """e_out = exp(z) elementwise, ~3e-7 rel accuracy. z, e_out: [126, n, 24]."""
    f32 = mybir.dt.float32
    shp = list(z.shape)
    t = pool.tile(shp, f32, tag="xp_t")
    nc.vector.tensor_scalar(out=t, in0=z, scalar1=INV_LN2, scalar2=MAGIC,
                            op0=mybir.AluOpType.mult, op1=mybir.AluOpType.add)
    nf = pool.tile(shp, f32, tag="xp_nf")
    nc.vector.tensor_scalar(out=nf, in0=t, scalar1=MAGIC, scalar2=None,
                            op0=mybir.AluOpType.subtract)
    r = pool.tile(shp, f32, tag="xp_r")
    nc.vector.scalar_tensor_tensor(out=r, in0=nf, scalar=-LN2_HI, in1=z,
                                   op0=mybir.AluOpType.mult, op1=mybir.AluOpType.add)
    nc.vector.scalar_tensor_tensor(out=r, in0=nf, scalar=-LN2_LO, in1=r,
                                   op0=mybir.AluOpType.mult, op1=mybir.AluOpType.add)
    # Horner: p = ((C7*r + C6)*r + C5)...
    p = pool.tile(shp, f32, tag="xp_p")
    nc.vector.tensor_scalar(out=p, in0=r, scalar1=EXPC[7], scalar2=EXPC[6],
                            op0=mybir.AluOpType.mult, op1=mybir.AluOpType.add)
    for k in range(5, -1, -1):
        nc.vector.tensor_tensor(out=p, in0=p, in1=r, op=mybir.AluOpType.mult)
        nc.vector.tensor_scalar_add(p, p, EXPC[k])
    # 2^n via exponent-field construction
    u = pool.tile(shp, f32, tag="xp_u")
    nc.vector.tensor_scalar(out=u, in0=nf, scalar1=127.0, scalar2=8388608.0,
                            op0=mybir.AluOpType.add, op1=mybir.AluOpType.mult)
    ui = pool.tile(shp, mybir.dt.int32, tag="xp_ui")
    nc.vector.tensor_copy(ui, u)
    nc.vector.tensor_tensor(out=e_out, in0=p, in1=ui[:, :, :].bitcast(f32),
                            op=mybir.AluOpType.mult)


def _build_program(debug=False):
    f32 = mybir.dt.float32
    u32 = mybir.dt.uint32
    Relu = mybir.ActivationFunctionType.Relu
    nc = bacc.Bacc()

    f16 = mybir.dt.float16
    featsh = nc.dram_tensor("featsh", [IMGS_PER_CORE, 128, 4, HWP], f16, kind="ExternalInput")
    featsl = nc.dram_tensor("featsl", [IMGS_PER_CORE, 128, 4, HWP], f16, kind="ExternalInput")
    w1th = nc.dram_tensor("w1th", [128, 4 * 9 * 256], f16, kind="ExternalInput")
    w1tl = nc.dram_tensor("w1tl", [128, 4 * 9 * 256], f16, kind="ExternalInput")
    w2th = nc.dram_tensor("w2th", [128, 2 * 9 * 128], f16, kind="ExternalInput")
    w2tl = nc.dram_tensor("w2tl", [128, 2 * 9 * 128], f16, kind="ExternalInput")
    w3th = nc.dram_tensor("w3th", [128, 1 * 9 * 64], f16, kind="ExternalInput")
    w3tl = nc.dram_tensor("w3tl", [128, 1 * 9 * 64], f16, kind="ExternalInput")
    wot = nc.dram_tensor("wot", [64, 45], f32, kind="ExternalInput")
    b1d = nc.dram_tensor("b1d", [128, 2], f32, kind="ExternalInput")
    b2d = nc.dram_tensor("b2d", [128, 1], f32, kind="ExternalInput")
    b3d = nc.dram_tensor("b3d", [64, 1], f32, kind="ExternalInput")
    bod = nc.dram_tensor("bod", [45, 1], f32, kind="ExternalInput")
    pmd = nc.dram_tensor("pmd", [PP, 1], f32, kind="ExternalInput")
    awd = nc.dram_tensor("awd", [PP, 1], f32, kind="ExternalInput")
    ahd = nc.dram_tensor("ahd", [PP, 1], f32, kind="ExternalInput")
    identd = nc.dram_tensor("identd", [128, 128], f32, kind="ExternalInput")

    boxes_o = nc.dram_tensor("boxes_o", [IMGS_PER_CORE, 4 * MAX_DET], f32, kind="ExternalOutput")
    gmlog_o = nc.dram_tensor("gmlog_o", [IMGS_PER_CORE, MAX_DET], f32, kind="ExternalOutput")
    if debug:
        p45_o = nc.dram_tensor("p45_o", [IMGS_PER_CORE, 45, HW], f32, kind="ExternalOutput")
        pool_o = nc.dram_tensor("pool_o", [IMGS_PER_CORE, PP, POOLW], f32, kind="ExternalOutput")
        gi_o = nc.dram_tensor("gi_o", [IMGS_PER_CORE, PP, POOLW], u32, kind="ExternalOutput")
        x9_o = nc.dram_tensor("x9_o", [IMGS_PER_CORE, PP, 9, POOLW], f32, kind="ExternalOutput")

    with TileContext(nc) as tc:
        with (
            tc.tile_pool(name="wp", bufs=1) as wp,
            tc.tile_pool(name="act", bufs=1) as actp,
            tc.tile_pool(name="nms", bufs=3) as nmsp,
            tc.tile_pool(name="xp", bufs=2) as xpp,
            tc.tile_pool(name="cpsp", bufs=4, space="PSUM") as cpsp,
            tc.tile_pool(name="npsp", bufs=3, space="PSUM") as npsp,
            tc.tile_pool(name="drp", bufs=1, space="DRAM") as drp,
        ):
            # ---- resident constants / weights (fp16 hi/lo)
            w1h = wp.tile([128, 4, 9, 256], f16)
            nc.sync.dma_start(out=w1h, in_=w1th[:, :].rearrange("p (k t o) -> p k t o", k=4, t=9))
            w1l = wp.tile([128, 4, 9, 256], f16)
            nc.sync.dma_start(out=w1l, in_=w1tl[:, :].rearrange("p (k t o) -> p k t o", k=4, t=9))
            w2h = wp.tile([128, 2, 9, 128], f16)
            nc.sync.dma_start(out=w2h, in_=w2th[:, :].rearrange("p (k t o) -> p k t o", k=2, t=9))
            w2l = wp.tile([128, 2, 9, 128], f16)
            nc.sync.dma_start(out=w2l, in_=w2tl[:, :].rearrange("p (k t o) -> p k t o", k=2, t=9))
            w3h = wp.tile([128, 1, 9, 64], f16)
            nc.sync.dma_start(out=w3h, in_=w3th[:, :].rearrange("p (k t o) -> p k t o", k=1, t=9))
            w3l = wp.tile([128, 1, 9, 64], f16)
            nc.sync.dma_start(out=w3l, in_=w3tl[:, :].rearrange("p (k t o) -> p k t o", k=1, t=9))
            wo = wp.tile([64, 45], f32)
            nc.sync.dma_start(out=wo, in_=wot[:, :])
            b1 = wp.tile([128, 2], f32)
            nc.sync.dma_start(out=b1, in_=b1d[:, :])
            b2 = wp.tile([128, 1], f32)
            nc.sync.dma_start(out=b2, in_=b2d[:, :])
            b3 = wp.tile([64, 1], f32)
            nc.sync.dma_start(out=b3, in_=b3d[:, :])
            bo = wp.tile([45, 1], f32)
            nc.sync.dma_start(out=bo, in_=bod[:, :])
            ident = wp.tile([128, 128], f32)
            nc.sync.dma_start(out=ident, in_=identd[:, :])
            pm224 = wp.tile([PP, 1], f32)
            nc.sync.dma_start(out=pm224, in_=pmd[:, :])
            awc = wp.tile([PP, 1], f32)
            nc.sync.dma_start(out=awc, in_=awd[:, :])
            ahc = wp.tile([PP, 1], f32)
            nc.sync.dma_start(out=ahc, in_=ahd[:, :])

            negt = wp.tile([PP, FF], f32)
            nc.vector.memset(negt, NEG)
            ones_col = wp.tile([PP, 1], f32)
            nc.vector.memset(ones_col, 1.0)
            ones_row = wp.tile([1, 128], f32)
            nc.vector.memset(ones_row, 1.0)
            ones_sq = wp.tile([PP, PP], f32)
            nc.vector.memset(ones_sq, 1.0)
            iota = wp.tile([PP, POOLW], u32)
            nc.gpsimd.iota(iota, pattern=[[0, POOLW]], base=0, channel_multiplier=FF)

            # ---- big activation tiles (shared across images; serial on PE)
            feath = actp.tile([128, 4, HWP], f16)
            featl = actp.tile([128, 4, HWP], f16)
            c1h = actp.tile([128, 2, HWP], f16)
            c1l = actp.tile([128, 2, HWP], f16)
            c2h = actp.tile([128, 1, HWP], f16)
            c2l = actp.tile([128, 1, HWP], f16)
            c3 = actp.tile([64, HW], f32)
            p45 = actp.tile([45, HW], f32)
            # zero the pad borders once (epilogues only ever write interiors)
            nc.vector.memset(c1h, 0.0)
            nc.vector.memset(c1l, 0.0)
            nc.vector.memset(c2h, 0.0)
            nc.vector.memset(c2l, 0.0)

            tbl = drp.tile([NCAND, 4], f32, tag="tbl")

            for img in range(IMGS_PER_CORE):
                # ---- convs (fp16 3-term split); per-ktile DMAs so conv1
                # can start on k-tile 0 while the rest stream in
                for kt in range(4):
                    nc.sync.dma_start(out=feath[:, kt, :], in_=featsh[img, :, kt, :])
                    nc.sync.dma_start(out=featl[:, kt, :], in_=featsl[img, :, kt, :])
                _conv_layer(nc, cpsp, xpp, feath[:, :, :], featl[:, :, :],
                            w1h[:, :, :, :], w1l[:, :, :, :], b1,
                            c1h[:, :, :], c1l[:, :, :],
                            n_kt=4, n_mt=2, oc=256, dst_padded=True)
                _conv_layer(nc, cpsp, xpp, c1h[:, :, :], c1l[:, :, :],
                            w2h[:, :, :, :], w2l[:, :, :, :], b2,
                            c2h[:, :, :], c2l[:, :, :],
                            n_kt=2, n_mt=1, oc=128, dst_padded=True)
                _conv_layer(nc, cpsp, xpp, c2h[:, :, :], c2l[:, :, :],
                            w3h[:, :, :, :], w3l[:, :, :, :], b3,
                            None, None,
                            n_kt=1, n_mt=1, oc=64, dst_padded=False, dst32=c3)
                _conv1x1_out(nc, cpsp, c3, wo, bo, p45)
                if debug:
                    nc.sync.dma_start(out=p45_o[img], in_=p45[:, :])

                # ---- write box-logit planes into the interleaved DRAM table
                for k in range(4):
                    nc.sync.dma_start(
                        out=tbl[:, k:k + 1],
                        in_=p45[9 * k:9 * (k + 1), :],
                    )

                # ---- score layout [126, 224] + conf filter (logit > 0)
                sfa = nmsp.tile([PP, FF], f32, tag="sfa")
                nc.sync.dma_start(out=sfa[:, :],
                                  in_=p45[36:45, :].rearrange("p (a b) -> p a b", b=FF))
                mneg = nmsp.tile([PP, FF], u32, tag="mneg")
                nc.vector.tensor_scalar(out=mneg, in0=sfa, scalar1=0.0, scalar2=None,
                                        op0=mybir.AluOpType.is_le)
                nc.vector.copy_predicated(sfa, mneg, negt[0:PP, :])

                # ---- per-partition top-24 pool (3 rounds of top-8)
                sp = nmsp.tile([PP, POOLW], f32, tag="sp")
                ii = nmsp.tile([PP, POOLW], u32, tag="ii")
                sfb = nmsp.tile([PP, FF], f32, tag="sfb")
                src = sfa
                for rnd in range(2):
                    nc.vector.max_with_indices(out_max=sp[:, 8 * rnd:8 * (rnd + 1)],
                                               out_indices=ii[:, 8 * rnd:8 * (rnd + 1)],
                                               in_=src)
                    if rnd < 1:
                        nc.vector.match_replace(out=sfb, in_to_replace=sp[:, 8 * rnd:8 * (rnd + 1)],
                                                in_values=src, imm_value=NEG)
                        src = sfb
                gi = nmsp.tile([PP, POOLW], u32, tag="gi")
                nc.vector.tensor_tensor(out=gi, in0=ii, in1=iota, op=mybir.AluOpType.add)

                # ---- gather tx,ty,tw,th rows for the pool (run-of-4 per row)
                g4 = nmsp.tile([PP, POOLW, 4], f32, tag="g4")
                for c in range(POOLW):
                    nc.gpsimd.indirect_dma_start(
                        out=g4[:, c, :], out_offset=None, in_=tbl[:, :],
                        in_offset=bass.IndirectOffsetOnAxis(ap=gi[:, c:c + 1], axis=0))
                # ---- derive gx, gy arithmetically: hw = (p % 14)*224 + f
                iif = nmsp.tile([PP, POOLW], f32, tag="iif")
                nc.vector.tensor_copy(iif, ii)
                hwf = nmsp.tile([PP, POOLW], f32, tag="hwf")
                nc.vector.tensor_scalar(out=hwf, in0=iif, scalar1=pm224, scalar2=None,
                                        op0=mybir.AluOpType.add)
                gyt = nmsp.tile([PP, POOLW], f32, tag="gyt")
                nc.vector.tensor_scalar(out=gyt, in0=hwf, scalar1=R56, scalar2=-0.499,
                                        op0=mybir.AluOpType.mult, op1=mybir.AluOpType.add)
                nc.vector.tensor_scalar_add(gyt, gyt, MAGIC)
                nc.vector.tensor_scalar(out=gyt, in0=gyt, scalar1=MAGIC, scalar2=None,
                                        op0=mybir.AluOpType.subtract)
                gxt = nmsp.tile([PP, POOLW], f32, tag="gxt")
                nc.vector.scalar_tensor_tensor(out=gxt, in0=gyt, scalar=-56.0, in1=hwf,
                                               op0=mybir.AluOpType.mult,
                                               op1=mybir.AluOpType.add)
                if debug:
                    nc.sync.dma_start(out=pool_o[img], in_=sp[:, :])
                    nc.sync.dma_start(out=gi_o[img], in_=gi[:, :])

                # ---- decode: exp/sigmoid + box geometry
                x9 = nmsp.tile([PP, 9, POOLW], f32, tag="x9")
                z = xpp.tile([PP, 4, POOLW], f32, tag="z")
                gtx = g4[:, :, :].rearrange("p w c -> p c w")
                nc.vector.tensor_scalar(out=z[:, 0:2, :], in0=gtx[:, 0:2, :],
                                        scalar1=-1.0, scalar2=None, op0=mybir.AluOpType.mult)
                nc.vector.tensor_copy(z[:, 2:4, :], gtx[:, 2:4, :])
                ez = xpp.tile([PP, 4, POOLW], f32, tag="ez")
                _exp_poly(nc, xpp, z[:, :, :], ez[:, :, :])
                # sigmoid for x,y
                d = xpp.tile([PP, 2, POOLW], f32, tag="d")
                nc.vector.tensor_scalar_add(d, ez[:, 0:2, :], 1.0)
                sg = xpp.tile([PP, 2, POOLW], f32, tag="sg")
                nc.vector.reciprocal(sg, d)
                # px,py = (sig + g)/56
                nc.vector.tensor_tensor(out=x9[:, 0, :], in0=sg[:, 0, :], in1=gxt,
                                        op=mybir.AluOpType.add)
                nc.vector.tensor_tensor(out=x9[:, 1, :], in0=sg[:, 1, :], in1=gyt,
                                        op=mybir.AluOpType.add)
                nc.vector.tensor_scalar(out=x9[:, 0:2, :], in0=x9[:, 0:2, :], scalar1=R56,
                                        scalar2=None, op0=mybir.AluOpType.mult)
                # pw,ph = min(exp * anc, 1.0), per-partition anchor scalars
                nc.vector.tensor_scalar(out=x9[:, 2, :], in0=ez[:, 2, :], scalar1=awc,
                                        scalar2=1.0, op0=mybir.AluOpType.mult,
                                        op1=mybir.AluOpType.min)
                nc.vector.tensor_scalar(out=x9[:, 3, :], in0=ez[:, 3, :], scalar1=ahc,
                                        scalar2=1.0, op0=mybir.AluOpType.mult,
                                        op1=mybir.AluOpType.min)
                # x1,y1 / -(x2,y2) / area
                h2 = xpp.tile([PP, 2, POOLW], f32, tag="h2")
                nc.vector.tensor_scalar(out=h2, in0=x9[:, 2:4, :], scalar1=0.5,
                                        scalar2=None, op0=mybir.AluOpType.mult)
                nc.vector.tensor_tensor(out=x9[:, 4:6, :], in0=x9[:, 0:2, :], in1=h2,
                                        op=mybir.AluOpType.subtract)
                t2 = xpp.tile([PP, 2, POOLW], f32, tag="t2")
                nc.vector.tensor_tensor(out=t2, in0=x9[:, 0:2, :], in1=h2,
                                        op=mybir.AluOpType.add)
                nc.vector.tensor_scalar(out=x9[:, 6:8, :], in0=t2, scalar1=-1.0,
                                        scalar2=None, op0=mybir.AluOpType.mult)
                e2 = xpp.tile([PP, 2, POOLW], f32, tag="e2")
                nc.vector.tensor_tensor(out=e2, in0=t2, in1=x9[:, 4:6, :],
                                        op=mybir.AluOpType.subtract)
                nc.vector.tensor_tensor(out=x9[:, 8, :], in0=e2[:, 0, :], in1=e2[:, 1, :],
                                        op=mybir.AluOpType.mult)
                if debug:
                    nc.sync.dma_start(out=x9_o[img], in_=x9[:, :, :])

                # ---- greedy NMS, 100 iterations
                out4 = nmsp.tile([1, 4 * MAX_DET], f32, tag="out4")
                gmlog = nmsp.tile([1, MAX_DET], f32, tag="gmlog")
                for it in range(MAX_DET):
                    ips = npsp.tile([128, 140], f32, tag="ips")
                    mT = ips[0:1, 0:PP]
                    gmB = ips[0:PP, 126:127]
                    w9b = ips[0:PP, 128:137]
                    mcol = nmsp.tile([PP, 1], f32, tag="mcol")
                    nc.vector.tensor_reduce(out=mcol, in_=sp, axis=mybir.AxisListType.X,
                                            op=mybir.AluOpType.max)
                    nc.tensor.transpose(mT, mcol, ident[0:PP, 0:PP])
                    nc.vector.tensor_reduce(out=gmlog[0:1, it:it + 1], in_=mT,
                                            axis=mybir.AxisListType.X, op=mybir.AluOpType.max)
                    nc.tensor.matmul(gmB, ones_row[:, 0:PP], gmlog[0:1, it:it + 1],
                                     start=True, stop=True)
                    oh = nmsp.tile([PP, POOLW], f32, tag="oh")
                    nc.vector.tensor_scalar(out=oh, in0=sp, scalar1=gmB, scalar2=None,
                                            op0=mybir.AluOpType.is_equal)
                    t9 = nmsp.tile([PP, 9, POOLW], f32, tag="t9")
                    nc.vector.tensor_tensor(out=t9, in0=x9,
                                            in1=_bc(oh[:, :], 9, None),
                                            op=mybir.AluOpType.mult)
                    r9 = nmsp.tile([PP, 9], f32, tag="r9")
                    nc.vector.tensor_reduce(out=r9, in_=t9, axis=mybir.AxisListType.X,
                                            op=mybir.AluOpType.add)
                    nc.tensor.matmul(w9b, ones_sq, r9, start=True, stop=True)
                    # suppression: M4 = max(candidate [x1,y1,-x2,-y2], winner's)
                    m4 = nmsp.tile([PP, 4, POOLW], f32, tag="m4")
                    nc.vector.tensor_tensor(out=m4, in0=x9[:, 4:8, :],
                                            in1=_bc(w9b[:, 4:8], 0, POOLW),
                                            op=mybir.AluOpType.max)
                    iw = nmsp.tile([PP, POOLW], f32, tag="iw")
                    nc.vector.scalar_tensor_tensor(out=iw, in0=m4[:, 2, :], scalar=-1.0,
                                                   in1=m4[:, 0, :], op0=mybir.AluOpType.mult,
                                                   op1=mybir.AluOpType.subtract)
                    ih = nmsp.tile([PP, POOLW], f32, tag="ih")
                    nc.vector.scalar_tensor_tensor(out=ih, in0=m4[:, 3, :], scalar=-1.0,
                                                   in1=m4[:, 1, :], op0=mybir.AluOpType.mult,
                                                   op1=mybir.AluOpType.subtract)
                    nc.vector.tensor_scalar(out=ih, in0=ih, scalar1=0.0, scalar2=None,
                                            op0=mybir.AluOpType.max)
                    inter = nmsp.tile([PP, POOLW], f32, tag="inter")
                    nc.vector.scalar_tensor_tensor(out=inter, in0=iw, scalar=0.0, in1=ih,
                                                   op0=mybir.AluOpType.max,
                                                   op1=mybir.AluOpType.mult)
                    # supp <=> 0.5*(area+areaw-inter+1e-16) < inter <=> area+areaw+1e-16 < 3*inter
                    uni = nmsp.tile([PP, POOLW], f32, tag="uni")
                    nc.vector.tensor_scalar(out=uni, in0=x9[:, 8, :], scalar1=w9b[:, 8:9],
                                            scalar2=1e-16, op0=mybir.AluOpType.add,
                                            op1=mybir.AluOpType.add)
                    supp = nmsp.tile([PP, POOLW], u32, tag="supp")
                    nc.vector.scalar_tensor_tensor(out=supp, in0=inter, scalar=3.0,
                                                   in1=uni, op0=mybir.AluOpType.mult,
                                                   op1=mybir.AluOpType.is_gt)
                    nc.vector.copy_predicated(sp, supp, negt[0:PP, 0:POOLW])
                    nc.scalar.copy(out=out4[0:1, 4 * it:4 * (it + 1)], in_=w9b[0:1, 0:4])

                nc.sync.dma_start(out=boxes_o[img:img+1, :], in_=out4[:, :])
                nc.sync.dma_start(out=gmlog_o[img:img+1, :], in_=gmlog[:, :])

    nc.compile()
    return nc


def _host_prep(inputs):
    f = {k: np.ascontiguousarray(np.asarray(v, dtype=np.float32)) for k, v in inputs.items()}

    def fold(w, b, g, be, m, v):
        inv = (g.astype(np.float64) / np.sqrt(v.astype(np.float64) + 1e-5)).astype(np.float32)
        wf = (w * inv[:, None, None, None]).astype(np.float32)
        bf = (b * inv + be - m * inv).astype(np.float32)
        return wf, bf

    w1f, b1f = fold(f["w1"], f["b1"], f["g1"], f["be1"], f["m1"], f["v1"])
    w2f, b2f = fold(f["w2"], f["b2"], f["g2"], f["be2"], f["m2"], f["v2"])
    w3f, b3f = fold(f["w3"], f["b3"], f["g3"], f["be3"], f["m3"], f["v3"])

    def lhsT(wf, n_kt, oc):
        arr = wf.reshape(oc, n_kt, 128, 9)          # [o, kt, ip, tap]
        arr = np.transpose(arr, (2, 1, 3, 0))        # [ip, kt, tap, o]
        arr = np.ascontiguousarray(arr.reshape(128, n_kt * 9 * oc))
        h = arr.astype(np.float16)
        l = (arr - h.astype(np.float32)).astype(np.float16)
        return h, l

    w1Th, w1Tl = lhsT(w1f, 4, 256)
    w2Th, w2Tl = lhsT(w2f, 2, 128)
    w3Th, w3Tl = lhsT(w3f, 1, 64)

    wo = f["wo"][:, :, 0, 0]                         # [54, 64]
    woT = np.zeros((64, 45), np.float32)
    boP = np.zeros((45, 1), np.float32)
    for c in range(5):
        for a in range(A):
            woT[:, c * 9 + a] = wo[a * 6 + c]
            boP[c * 9 + a, 0] = f["bo"][a * 6 + c]

    b1P = np.ascontiguousarray(b1f.reshape(2, 128).T)         # [128, 2]
    b2P = np.ascontiguousarray(b2f.reshape(1, 128).T)
    b3P = b3f.reshape(64, 1).copy()

    feats_pad = np.zeros((B, 4, 128, HP, WP), np.float32)
    feats_pad[:, :, :, 1:57, 1:57] = f["features"].reshape(B, 4, 128, H, W)
    feats_pad = np.ascontiguousarray(
        feats_pad.reshape(B, 4, 128, HWP).transpose(0, 2, 1, 3))
    feats_h = feats_pad.astype(np.float16)
    feats_l = (feats_pad - feats_h.astype(np.float32)).astype(np.float16)

    # per-partition constants: partition p covers i in [p*224, (p+1)*224),
    # anchor a = p//14, hw-offset = (p%14)*224
    pidx = np.arange(PP)
    anc = np.array([[np.float32(s * math.sqrt(r) / 224.0),
                     np.float32(s / math.sqrt(r) / 224.0)]
                    for s in [32.0, 64.0, 128.0] for r in [0.5, 1.0, 2.0]], np.float32)
    pm = ((pidx % 14) * 224).astype(np.float32)[:, None]
    aw = anc[pidx // 14, 0][:, None].copy()
    ah = anc[pidx // 14, 1][:, None].copy()

    shared = {
        "w1th": w1Th, "w1tl": w1Tl, "w2th": w2Th, "w2tl": w2Tl,
        "w3th": w3Th, "w3tl": w3Tl, "wot": woT,
        "b1d": b1P, "b2d": b2P, "b3d": b3P, "bod": boP,
        "pmd": pm, "awd": aw, "ahd": ah,
        "identd": np.eye(128, dtype=np.float32),
    }
    in_maps = []
    for c in range(N_CORES):
        m = dict(shared)
        m["featsh"] = np.ascontiguousarray(feats_h[c * IMGS_PER_CORE:(c + 1) * IMGS_PER_CORE])
        m["featsl"] = np.ascontiguousarray(feats_l[c * IMGS_PER_CORE:(c + 1) * IMGS_PER_CORE])
        in_maps.append(m)
    return in_maps


def kernel(debug=False, trace=False, **inputs):
    key = bool(debug)
    if key not in _PROGRAM_CACHE:
        _PROGRAM_CACHE[key] = _build_program(debug=debug)
    nc = _PROGRAM_CACHE[key]
    in_maps = _host_prep(inputs)
    out = run_bass_kernel_spmd(nc, in_maps, core_ids=list(range(N_CORES)), trace=trace)
    res = out.results
    det_boxes = np.zeros((B, MAX_DET, 4), np.float32)
    det_valid = np.zeros((B, MAX_DET), bool)
    dbg = []
    for c in range(N_CORES):
        r = res[c]
        bx = r["boxes_o"].reshape(IMGS_PER_CORE, MAX_DET, 4)
        gm = r["gmlog_o"]
        ok = gm > np.float32(-5e29)
        bx = np.where(ok[:, :, None], bx, 0.0).astype(np.float32)
        det_boxes[c * IMGS_PER_CORE:(c + 1) * IMGS_PER_CORE] = bx
        det_valid[c * IMGS_PER_CORE:(c + 1) * IMGS_PER_CORE] = ok
        if debug:
            dbg.append(r)
    if debug:
        kernel.debug_results = dbg
        kernel.exec_time_ns = out.exec_time_ns
        return det_boxes, det_valid
    if trace:
        kernel.exec_time_ns = out.exec_time_ns
    return det_boxes, det_valid
